# revision 36
# baseline (speedup 1.0000x reference)
"""Trainium2 Bass kernel for nn_DglAggregator (GNN message passing).

Strategy (8 NeuronCores, SPMD, one uniform program, per-core data):
- Targets are partitioned across cores balanced by stage-1 edge count; each
  core owns its targets' items and ALL stage-1 edges pointing at those items,
  so no cross-core communication is needed.
- Stage 1 (item->item segment softmax + weighted sum): items laid out in
  contiguous "islot" order; windows = consecutive islot ranges (<=128 islots,
  <=1024 edges); 8 windows per batch (8192 edge ranks). Per batch the edge
  source rows come from ONE bf16 dma_gather ([edge, d] rank order) out of a
  per-batch deduplicated region (<=8192 rows, int16-addressable) of a
  host-relaid table; the transposed layout [d, edge] needed by the score
  matmul is produced on-chip (PE transposes through PSUM + Act/DVE copies).
  Scores: S[e,s] = Xs_e . (h_v[dst_s] * pi) on TensorE against per-window
  dst-slot columns (no per-edge dst gather; pi is folded in on-chip).
  Softmax: a bias tile B0[e,s] = (seg(e)!=s) * -3e4 (one fused two-op
  tensor_scalar on DVE) is matmul-accumulated into the score PSUM, so one
  batched exp(S+B0) on the Activation engine directly yields the masked
  softmax weights, which feed the ft/den matmuls as lhsT. Max-subtraction
  is skipped (|score| small, exact in f32). Per-window softmax denominators
  accumulate as a second PSUM group AFTER the ft group (PSUM accumulation
  groups must never interleave within a bank on real hardware).
- Stage 2 (item->target): masked-matmul pattern with bf16 operands; ft rows
  come via normal + transpose dma_gathers of the stage-1 output table. e2 is
  computed in transposed orientation (lhsT = qw, rhs = transposed ft/hp), so
  the per-edge weight w = <e2, f[dst]> becomes a row-select of W = e2T^T fT,
  done by one fused scalar_tensor_tensor with accum_out; no f-table DRAM
  round-trip or gather. Degree normalization (1/max(deg,1)) is host graph
  metadata. Stage-2 windows are emitted in small chunks interleaved between
  stage-1 batches as soon as their ft rows exist (cuts), hiding the tail.
- Numeric tables (h_v/h_p/h_t) are staged in bfloat16; all arithmetic
  (pi scaling, matmuls, softmax, tanh, means) runs on the NeuronCores with
  f32 PSUM accumulation. Host work is index math, row permutation/layout
  of input tables, and dtype staging.

kernel(**inputs) accepts the FULL unsharded inputs and returns the FULL
[N_TGT, 128] float32 output.
"""
import numpy as np
import ml_dtypes

BF16 = np.dtype(ml_dtypes.bfloat16)

P = 128          # partitions / tile edge
D = 128          # feature dim
NCORES = 8
WE1 = 1024       # stage-1 window edge capacity (8 tiles)
WS1 = 128        # stage-1 window slot capacity
WB = 8           # stage-1 windows per batch
RB = WB * WE1    # edge ranks per batch (8192)
TI2 = 50         # stage-2 tiles per window (6400 item slots)
WS2 = 128        # stage-2 window target capacity
GH = 4096        # gather granularity (half batch)
DEBUG_FT = False # expose stage-1 ft table as an output
_LAST_NC = None


def _wrap_idx16(idx: np.ndarray, cap: int) -> np.ndarray:
    """[n<=cap] -> [128, cap/16] int16 (j at [j%16, j//16], replicated x8)."""
    a = np.zeros(cap, np.int64)
    a[: idx.shape[0]] = idx
    assert cap % 16 == 0
    assert a.min() >= 0 and a.max() < 32768, (a.min(), a.max())
    blk = a.reshape(cap // 16, 16).T.astype(np.int16)
    return np.tile(blk, (8, 1))


def _interleave_f32(vals: np.ndarray, cap: int, fill: float) -> np.ndarray:
    """[n] -> [128, cap/128] f32 with value of rank r at [r%128, r//128]."""
    a = np.full(cap, fill, np.float32)
    a[: vals.shape[0]] = vals
    return a.reshape(cap // P, P).T.copy()


def _pack_runs(run_sizes, max_runs, max_total):
    """Greedy pack consecutive runs into groups of whole runs, <=max_runs
    runs and <=max_total total size. Returns list of (start_run, n_runs)."""
    groups = []
    i, n = 0, len(run_sizes)
    while i < n:
        tot, j = 0, i
        while j < n and j - i < max_runs and tot + run_sizes[j] <= max_total:
            tot += run_sizes[j]
            j += 1
        assert j > i, f"run {i} of size {run_sizes[i]} exceeds {max_total}"
        groups.append((i, j - i))
        i = j
    return groups


def preprocess(h_v, h_p, h_t, int_src, int_dst, agg_dst):
    """All graph restructuring. Returns shared dims + per-core arrays."""
    NITEM = h_v.shape[0]
    NTGT = h_t.shape[0]
    int_src = int_src.astype(np.int64)
    int_dst = int_dst.astype(np.int64)
    item_tgt = agg_dst.astype(np.int64)       # item i -> target (agg_src=arange)
    h_v_bf = h_v.astype(BF16)
    h_p_bf = h_p.astype(BF16)
    h_t_bf = h_t.astype(BF16)

    # ---- target -> core, balanced by stage-1 edge load ----
    deg_int = np.bincount(int_dst, minlength=NITEM)
    t_edges = np.bincount(item_tgt, weights=deg_int.astype(np.float64),
                          minlength=NTGT)
    t_items = np.bincount(item_tgt, minlength=NTGT)
    tgt_core = np.zeros(NTGT, np.int64)
    load = np.zeros(NCORES)
    for t in np.argsort(-t_edges, kind="stable"):
        c = int(np.argmin(load))
        tgt_core[t] = c
        load[c] += t_edges[t] + 0.5 * t_items[t]
    item_core = tgt_core[item_tgt]

    cores = []
    for c in range(NCORES):
        tlist = np.where(tgt_core == c)[0]
        items = np.where(item_core == c)[0]
        items = items[np.lexsort((items, item_tgt[items]))]
        cores.append({"targets": tlist, "items": items})

    # ---- stage-2 windows (whole targets, <=WS2 targets, <=TI2*128 islots) ----
    for c in range(NCORES):
        st = cores[c]
        st["w2groups"] = _pack_runs(t_items[st["targets"]], WS2, TI2 * P)
    W2 = max(len(st["w2groups"]) for st in cores)
    NI = W2 * TI2 * P

    for c in range(NCORES):
        st = cores[c]
        tl, items = st["targets"], st["items"]
        it_item = np.full(NI, -1, np.int64)        # islot -> global item
        it_tgtloc = np.full(NI, -1.0, np.float32)  # islot -> window-local tgt
        it_tslot = np.zeros(NI, np.int64)          # islot -> global tgt slot
        twin = np.full((W2, WS2), -1, np.int64)    # window -> global targets
        ipos = 0
        for w2, (t0, ntgt) in enumerate(st["w2groups"]):
            base = w2 * TI2 * P
            off = 0
            for k in range(ntgt):
                t = tl[t0 + k]
                cnt = int(t_items[t])
                sl = slice(base + off, base + off + cnt)
                it_item[sl] = items[ipos: ipos + cnt]
                it_tgtloc[sl] = k
                it_tslot[sl] = w2 * WS2 + k
                twin[w2, k] = t
                ipos += cnt
                off += cnt
        assert ipos == len(items)
        st["it_item"] = it_item
        st["it_tgtloc"] = it_tgtloc
        st["it_tslot"] = it_tslot
        st["twin"] = twin
        islot_of = np.full(NITEM, -1, np.int64)
        real = it_item >= 0
        islot_of[it_item[real]] = np.where(real)[0]
        st["islot_of"] = islot_of

    # ---- stage-1 windows: consecutive islot ranges ----
    for c in range(NCORES):
        st = cores[c]
        emask = item_core[int_dst] == c
        es = int_src[emask]
        ed = st["islot_of"][int_dst[emask]]
        o = np.argsort(ed, kind="stable")
        st["e_src"], st["e_dst"] = es[o], ed[o]
        cnt = np.bincount(st["e_dst"], minlength=NI)
        st["w1groups"] = _pack_runs(cnt, WS1, WE1)   # (islot0, nislots)
        st["islot_cnt"] = cnt
    W1 = max(len(st["w1groups"]) for st in cores)
    W1 = ((W1 + WB - 1) // WB) * WB
    B1 = W1 // WB
    assert W1 * WS1 <= 32768, f"ft table too big for int16: W1={W1}"

    for c in range(NCORES):
        st = cores[c]
        es, ed, cnt = st["e_src"], st["e_dst"], st["islot_cnt"]
        estart = np.concatenate([[0], np.cumsum(cnt)])
        wsrc = np.zeros((W1, WE1), np.int64)
        wseg = np.full((W1, WE1), -1.0, np.float32)
        ft_slot = np.zeros(NI, np.int64)
        wbase = np.full(W1, NI, np.int64)            # pad windows -> zero cols
        for w, (i0, ni) in enumerate(st["w1groups"]):
            e0, e1 = estart[i0], estart[i0 + ni]
            ne = int(e1 - e0)
            assert ne <= WE1 and ni <= WS1
            wsrc[w, :ne] = es[e0:e1]
            wseg[w, :ne] = (ed[e0:e1] - i0).astype(np.float32)
            ft_slot[i0: i0 + ni] = w * WS1 + np.arange(ni)
            wbase[w] = i0
        st["wsrc"], st["wseg"] = wsrc, wseg
        st["ft_slot"] = ft_slot
        st["wbase"] = wbase

    # ---- per-batch gather regions + index/seg arrays ----
    for c in range(NCORES):
        st = cores[c]
        hv2 = np.zeros((B1 * RB, D), BF16)
        g2 = np.zeros((B1, P, RB // 16), np.int16)
        seg = np.full((B1, P, RB // P), -1.0, np.float32)
        for b in range(B1):
            wins = slice(b * WB, (b + 1) * WB)
            src = st["wsrc"][wins].reshape(-1)
            sg = st["wseg"][wins].reshape(-1)
            real = sg >= 0
            uniq = np.unique(src[real])
            if uniq.size == 0:
                uniq = np.array([0], np.int64)
            assert uniq.size <= RB
            hv2[b * RB: b * RB + uniq.size] = h_v_bf[uniq]
            pos = np.zeros(RB, np.int64)
            pos[real] = np.searchsorted(uniq, src[real])
            g2[b] = _wrap_idx16(pos, RB)
            seg[b] = _interleave_f32(sg, RB, -1.0)
        st["hv2"], st["g2"], st["seg"] = hv2, g2, seg

        # window-padded dst table [D, W1*128] (col w*128+s = h_v[islot base+s])
        colitem = np.full(W1 * WS1, -1, np.int64)
        for w, (i0, ni) in enumerate(st["w1groups"]):
            colitem[w * WS1: w * WS1 + ni] = st["it_item"][i0: i0 + ni]
        hvlTw = np.zeros((D, W1 * WS1), BF16)
        cr = colitem >= 0
        hvlTw[:, cr] = h_v_bf[colitem[cr]].T
        st["hvlTw"] = hvlTw

    # ---- stage-2 gather/meta arrays + tables ----
    for c in range(NCORES):
        st = cores[c]
        it_item = st["it_item"]
        real = it_item >= 0
        st["ftg"] = _wrap_idx16(st["ft_slot"], NI)
        st["fexp"] = _wrap_idx16(st["it_tslot"], NI)
        tl = np.zeros((W2, P, TI2), np.float32)
        for w2 in range(W2):
            tl[w2] = _interleave_f32(
                st["it_tgtloc"][w2 * TI2 * P: (w2 + 1) * TI2 * P], TI2 * P,
                -1.0)
        st["tgtloc"] = tl
        # host-side degree normalization: deg[t] is graph structure
        r2 = np.ones((W2, P, 1), np.float32)
        tw2 = st["twin"]
        for w2 in range(W2):
            sel = tw2[w2] >= 0
            r2[w2, sel, 0] = 1.0 / np.maximum(t_items[tw2[w2][sel]], 1)
        st["rec2"] = r2
        hpT = np.zeros((D, NI), BF16)
        hpT[:, real] = h_p_bf[it_item[real]].T
        st["hpT"] = hpT
        htT = np.zeros((D, W2 * WS2), BF16)
        tw = st["twin"].reshape(-1)
        htT[:, tw >= 0] = h_t_bf[tw[tw >= 0]].T
        st["htT"] = htT

    # earliest stage-1 batch after which each stage-2 window's ft rows exist
    cuts = []
    for w2 in range(W2):
        E = (w2 + 1) * TI2 * P
        c_max = 0
        for c in range(NCORES):
            lastw = max(w for w, (i0, ni) in enumerate(cores[c]["w1groups"])
                        if i0 < E)
            c_max = max(c_max, lastw // WB)
        cuts.append(c_max)
    cuts = [max(cuts[: i + 1]) for i in range(W2)]
    cuts[W2 - 1] = B1 - 1

    dims = {"NI": NI, "W1": W1, "B1": B1, "W2": W2, "cuts": cuts,
            "NITEM": NITEM, "NTGT": NTGT}
    return dims, cores


# ======================= device program =======================

def build_program(dims):
    import concourse.bacc as bacc
    import concourse.mybir as mybir
    import concourse.tile as tile

    f32 = mybir.dt.float32
    bf16 = mybir.dt.bfloat16
    i16 = mybir.dt.int16
    Alu = mybir.AluOpType
    Act = mybir.ActivationFunctionType
    Ax = mybir.AxisListType

    NI, W1, B1, W2 = (dims[k] for k in ("NI", "W1", "B1", "W2"))
    FTC = W1 * WS1                     # dst-table columns / ft rows
    NW = TI2 * P                       # islots per stage-2 window

    nc = bacc.Bacc("TRN2", target_bir_lowering=False, debug=False,
                   num_devices=NCORES)
    # inputs
    hv2 = nc.dram_tensor("hv2", [B1 * RB, D], bf16, kind="ExternalInput")
    hvlTw = nc.dram_tensor("hvlTw", [D, FTC], bf16, kind="ExternalInput")
    hpT = nc.dram_tensor("hpT", [D, NI], bf16, kind="ExternalInput")
    htT = nc.dram_tensor("htT", [D, W2 * WS2], bf16, kind="ExternalInput")
    qw = nc.dram_tensor("qw", [2 * D, D], f32, kind="ExternalInput")
    rw = nc.dram_tensor("rw", [2 * D, D], f32, kind="ExternalInput")
    pic = nc.dram_tensor("pic", [D, 1], f32, kind="ExternalInput")
    iotab = nc.dram_tensor("iotab", [P, P], bf16, kind="ExternalInput")
    ident = nc.dram_tensor("ident", [P, P], f32, kind="ExternalInput")
    g2d = nc.dram_tensor("g2d", [B1, P, RB // 16], i16, kind="ExternalInput")
    segd = nc.dram_tensor("segd", [B1, P, RB // P], f32, kind="ExternalInput")
    ftgd = nc.dram_tensor("ftgd", [P, NI // 16], i16, kind="ExternalInput")
    tgtlocd = nc.dram_tensor("tgtlocd", [W2, P, TI2], f32, kind="ExternalInput")
    rec2d = nc.dram_tensor("rec2d", [W2, P, 1], f32, kind="ExternalInput")
    # output
    outd = nc.dram_tensor("out", [W2 * WS2, D], f32, kind="ExternalOutput")
    # internal scratch
    ftd = nc.dram_tensor("ft", [FTC, D], bf16,
                         kind="ExternalOutput" if DEBUG_FT else "Internal")

    with tile.TileContext(nc) as tc:
        with (
            tc.tile_pool(name="consts", bufs=1) as cp,
            tc.tile_pool(name="weights", bufs=1) as wp,
        ):
            iota_t = cp.tile([P, P], bf16)
            nc.sync.dma_start(out=iota_t[:], in_=iotab[:])
            ident_t = cp.tile([P, P], f32)
            nc.sync.dma_start(out=ident_t[:], in_=ident[:])
            ident_b = cp.tile([P, P], bf16)
            nc.scalar.activation(out=ident_b[:], in_=ident_t[:], func=Act.Copy)
            ones_b = cp.tile([P, 1], bf16)
            nc.vector.memset(ones_b[:], 1.0)
            pi_t = cp.tile([D, 1], f32)
            nc.sync.dma_start(out=pi_t[:], in_=pic[:])
            # weights: load f32, cast to bf16 on device
            qwf = wp.tile([P, 2, D], f32)
            nc.sync.dma_start(out=qwf[:, 0, :], in_=qw[0:D, :])
            nc.sync.dma_start(out=qwf[:, 1, :], in_=qw[D: 2 * D, :])
            qwb_t = wp.tile([P, 2, D], bf16)
            nc.scalar.activation(out=qwb_t[:], in_=qwf[:], func=Act.Copy)
            rwf = wp.tile([P, 2, D], f32)
            nc.sync.dma_start(out=rwf[:, 0, :], in_=rw[0:D, :])
            nc.sync.dma_start(out=rwf[:, 1, :], in_=rw[D: 2 * D, :])
            rwb_t = wp.tile([P, 2, D], bf16)
            nc.scalar.activation(out=rwb_t[:], in_=rwf[:], func=Act.Copy)

            # ---- P1 batches with stage-2 windows interleaved at cuts ----
            cuts = dims["cuts"]
            from contextlib import ExitStack
            with ExitStack() as stack:
                pool = lambda *a, **k: stack.enter_context(
                    tc.tile_pool(*a, **k))
                ip1 = pool(name="idx1", bufs=3)
                gp = pool(name="gat", bufs=4)
                tp = pool(name="xsT1", bufs=10)
                xp = pool(name="ex1", bufs=8)
                mp = pool(name="mx1", bufs=14)
                sm = pool(name="sm1", bufs=12)
                fsp = pool(name="fts", bufs=3)
                ip2 = pool(name="idx2", bufs=1)
                bg = pool(name="big2", bufs=1)
                wk2 = pool(name="wk2", bufs=10)
                xp2 = pool(name="ex2", bufs=10)
                sm2 = pool(name="sm2", bufs=8)
                psS = pool(name="psS", bufs=2, space="PSUM")
                psT = pool(name="psT", bufs=2, space="PSUM")
                psF = pool(name="psF", bufs=2, space="PSUM")
                ppA = pool(name="psA", bufs=1, space="PSUM")
                ppB = pool(name="psB", bufs=1, space="PSUM")
                ftgt = ip2.tile([P, NI // 16], i16, tag="ftg")
                nc.sync.dma_start(out=ftgt[:], in_=ftgd[:])

                def s1_load(b):
                    g2t = ip1.tile([P, RB // 16], i16, tag="g2")
                    nc.sync.dma_start(out=g2t[:], in_=g2d[b])
                    segt = ip1.tile([P, RB // P], f32, tag="seg")
                    nc.sync.dma_start(out=segt[:], in_=segd[b])
                    xdw0 = ip1.tile([P, WB * WS1], bf16, tag="xdw0")
                    nc.sync.dma_start(
                        out=xdw0[:],
                        in_=hvlTw[:, b * WB * WS1: (b + 1) * WB * WS1])
                    xdw = ip1.tile([P, WB * WS1], bf16, tag="xdw")
                    nc.vector.tensor_scalar_mul(xdw[:], xdw0[:], pi_t[:])
                    return g2t, segt, xdw

                def s1_batch(b, loaded):
                    g2t, segt, xdw = loaded
                    ftstage = fsp.tile([P, WB, D], bf16, tag="fts")
                    for h in range(2):
                        xs = gp.tile([P, GH // P, D], bf16, tag="xs")
                        nc.gpsimd.dma_gather(
                            out_ap=xs[:], in_ap=hv2[b * RB: (b + 1) * RB],
                            idxs_ap=g2t[:, h * GH // 16: (h + 1) * GH // 16],
                            num_idxs=GH, num_idxs_reg=GH, elem_size=D,
                            single_packet=False)
                        for wl in range(WB // 2):
                            w = h * (WB // 2) + wl
                            fdp = psF.tile([P, D + 1], f32,
                                           space="PSUM", tag="ftden")
                            ftp = fdp[:, 0:D]
                            denp = fdp[:, D: D + 1]
                            exg = []
                            for g in range(2):
                                # transpose 4 tiles: [e,d] -> [d,e]
                                trp = psT.tile([P, 4 * P], bf16, space="PSUM",
                                               tag="tr")
                                for j in range(4):
                                    t = wl * 8 + g * 4 + j   # tile in half
                                    nc.tensor.transpose(
                                        out=trp[:, j * P: (j + 1) * P],
                                        in_=xs[:, t, :], identity=ident_b[:])
                                xsT = tp.tile([P, 4 * P], bf16, tag="xsT")
                                if (wl * 2 + g) % 2 == 0:
                                    nc.scalar.activation(out=xsT[:],
                                                         in_=trp[:],
                                                         func=Act.Copy)
                                else:
                                    nc.vector.tensor_copy(out=xsT[:],
                                                          in_=trp[:])
                                sp = psS.tile([P, 4 * P], f32, space="PSUM",
                                              tag="sp")
                                for j in range(4):
                                    tb = w * 8 + g * 4 + j   # tile in batch
                                    # B0 = (iota != seg) * -30000: exp(S+B0)
                                    # is the masked softmax weight directly
                                    b0 = mp.tile([P, P], bf16, tag="b0")
                                    nc.vector.tensor_scalar(
                                        out=b0[:], in0=iota_t[:],
                                        scalar1=segt[:, tb: tb + 1],
                                        scalar2=-30000.0,
                                        op0=Alu.not_equal, op1=Alu.mult)
                                    nc.tensor.matmul(
                                        out=sp[:, j * P: (j + 1) * P],
                                        lhsT=xsT[:, j * P: (j + 1) * P],
                                        rhs=xdw[:, w * WS1: (w + 1) * WS1],
                                        start=True, stop=False)
                                    nc.tensor.matmul(
                                        out=sp[:, j * P: (j + 1) * P],
                                        lhsT=ident_b[:], rhs=b0[:],
                                        start=False, stop=True)
                                ex = xp.tile([P, 4 * P], bf16, tag="ex")
                                nc.scalar.activation(out=ex[:], in_=sp[:],
                                                     func=Act.Exp)
                                exg.append(ex)
                                for j in range(4):
                                    t = wl * 8 + g * 4 + j
                                    i = g * 4 + j            # tile in window
                                    nc.tensor.matmul(
                                        out=ftp,
                                        lhsT=ex[:, j * P: (j + 1) * P],
                                        rhs=xs[:, t, :],
                                        start=(i == 0), stop=(i == 7))
                            # den group AFTER the ft group: accumulation
                            # groups must not interleave within a PSUM bank
                            for i in range(8):
                                nc.tensor.matmul(
                                    out=denp,
                                    lhsT=exg[i // 4][:, (i % 4) * P:
                                                     (i % 4 + 1) * P],
                                    rhs=ones_b[:],
                                    start=(i == 0), stop=(i == 7))
                            denc = sm.tile([P, 1], f32, tag="denc")
                            nc.vector.tensor_scalar_max(denc[:], denp,
                                                        1e-30)
                            rec = sm.tile([P, 1], f32, tag="rec")
                            nc.vector.reciprocal(rec[:], denc[:])
                            nc.vector.tensor_scalar_mul(
                                ftstage[:, w, :], ftp, rec[:])
                    nc.scalar.dma_start(
                        out=ftd[b * WB * WS1: (b + 1) * WB * WS1, :].rearrange(
                            "(w p) d -> p w d", p=P),
                        in_=ftstage[:])

                def s2_c0(w2, st2):
                    hpt = bg.tile([P, NW], bf16, tag="hpt")
                    nc.sync.dma_start(out=hpt[:],
                                      in_=hpT[:, w2 * NW: (w2 + 1) * NW])
                    tlt = sm2.tile([P, TI2], f32, tag="tlt")
                    nc.sync.dma_start(out=tlt[:], in_=tgtlocd[w2])
                    ftg = bg.tile([P, TI2, D], bf16, tag="ftgw")
                    ftgT = bg.tile([P, 1, NW], bf16, tag="ftgTw")
                    for o0, n in ((0, 4096), (4096, NW - 4096)):
                        o = w2 * NW + o0
                        nc.gpsimd.dma_gather(
                            out_ap=ftg[:, o0 // P: (o0 + n) // P, :],
                            in_ap=ftd[:],
                            idxs_ap=ftgt[:, o // 16: (o + n) // 16],
                            num_idxs=n, num_idxs_reg=n, elem_size=D,
                            single_packet=False)
                        nc.gpsimd.dma_gather(
                            out_ap=ftgT[:, :, o0: o0 + n],
                            in_ap=ftd[:],
                            idxs_ap=ftgt[:, o // 16: (o + n) // 16],
                            num_idxs=n, num_idxs_reg=n, elem_size=D,
                            single_packet=False, transpose=True)
                    st2.update(hpt=hpt, tlt=tlt, ftg=ftg, ftgT=ftgT)

                def s2_c1(w2, st2):
                    hpt, tlt, ftg, ftgT = (st2[k] for k in
                                           ("hpt", "tlt", "ftg", "ftgT"))
                    # one PSUM bank per window: mean | f | out, with
                    # strictly sequential accumulation groups (groups must
                    # not interleave within a PSUM bank)
                    apo = ppA.tile([P, 3 * D], f32, space="PSUM", tag="apo")
                    meanp = apo[:, 0:D]
                    fp = apo[:, D: 2 * D]
                    outp = apo[:, 2 * D: 3 * D]
                    rec2 = sm2.tile([P, 1], f32, tag="rec2")
                    nc.sync.dma_start(out=rec2[:], in_=rec2d[w2])
                    st2.update(apo=apo, meanp=meanp, fp=fp, outp=outp,
                               rec2=rec2)
                    # sweep A: mean (first half); deg comes from the host
                    for i in range(TI2 // 2):
                        mask = wk2.tile([P, P], bf16, tag="maskA")
                        nc.vector.tensor_scalar(
                            out=mask[:], in0=iota_t[:],
                            scalar1=tlt[:, i: i + 1], scalar2=None,
                            op0=Alu.is_equal)
                        nc.tensor.matmul(out=meanp, lhsT=mask[:],
                                         rhs=ftg[:, i, :],
                                         start=(i == 0), stop=(i == TI2 - 1))
                def s2_c2(w2, st2):
                    hpt, tlt, ftg, ftgT = (st2[k] for k in
                                           ("hpt", "tlt", "ftg", "ftgT"))
                    meanp, fp, rec2 = (st2[k] for k in
                                       ("meanp", "fp", "rec2"))
                    for i in range(TI2 // 2, TI2):
                        mask = wk2.tile([P, P], bf16, tag="maskA")
                        nc.vector.tensor_scalar(
                            out=mask[:], in0=iota_t[:],
                            scalar1=tlt[:, i: i + 1], scalar2=None,
                            op0=Alu.is_equal)
                        nc.tensor.matmul(out=meanp, lhsT=mask[:],
                                         rhs=ftg[:, i, :],
                                         start=(i == 0), stop=(i == TI2 - 1))
                    mean_sb = wk2.tile([P, D], f32, tag="mean_sb")
                    nc.vector.tensor_scalar_mul(mean_sb[:], meanp, rec2[:])
                    trx = ppB.tile([P, 4 * P], f32, space="PSUM", tag="big")
                    nc.tensor.transpose(out=trx[:, 0:P], in_=mean_sb[:],
                                        identity=ident_t[:])
                    meanT = wk2.tile([P, P], bf16, tag="meanT")
                    nc.scalar.activation(out=meanT[:], in_=trx[:, 0:P],
                                         func=Act.Copy)
                    htt = wk2.tile([P, P], bf16, tag="htt")
                    nc.sync.dma_start(out=htt[:],
                                      in_=htT[:, w2 * WS2: (w2 + 1) * WS2])
                    nc.tensor.matmul(out=fp, lhsT=htt[:], rhs=rwb_t[:, 0, :],
                                     start=True, stop=False)
                    nc.tensor.matmul(out=fp, lhsT=meanT[:],
                                     rhs=rwb_t[:, 1, :],
                                     start=False, stop=True)
                    # fT = f transposed [d, tgt] (for W = e2T^T @ fT)
                    f_sb = wk2.tile([P, D], f32, tag="f_sb")
                    nc.vector.tensor_copy(out=f_sb[:], in_=fp)
                    trf = ppB.tile([P, 4 * P], f32, space="PSUM", tag="big")
                    nc.tensor.transpose(out=trf[:, 0:P], in_=f_sb[:],
                                        identity=ident_t[:])
                    fTb = wk2.tile([P, P], bf16, tag="fTb")
                    nc.scalar.activation(out=fTb[:], in_=trf[:, 0:P],
                                         func=Act.Copy)
                    st2.update(fTb=fTb)

                def s2_swb(w2, st2, g0s):
                    hpt, tlt, ftg, ftgT, fTb, outp = (st2[k] for k in
                        ("hpt", "tlt", "ftg", "ftgT", "fTb", "outp"))
                    # sweep B: e2T = tanh(qw^T [ft, hp]^T); W = e2T^T fT;
                    # wc[islot] = W[islot, tl(islot)] via fused mask+reduce
                    for g0 in g0s:
                        gn = min(4, TI2 - g0)
                        e2p = ppB.tile([P, 4 * P], f32, space="PSUM",
                                       tag="big")
                        for j in range(gn):
                            i = g0 + j
                            nc.tensor.matmul(
                                out=e2p[:, j * P: (j + 1) * P],
                                lhsT=qwb_t[:, 0, :],
                                rhs=ftgT[:, 0, i * P: (i + 1) * P],
                                start=True, stop=False)
                            nc.tensor.matmul(
                                out=e2p[:, j * P: (j + 1) * P],
                                lhsT=qwb_t[:, 1, :],
                                rhs=hpt[:, i * P: (i + 1) * P],
                                start=False, stop=True)
                        e2T = xp2.tile([P, 4 * P], bf16, tag="e2sb")
                        nc.scalar.activation(out=e2T[:, : gn * P],
                                             in_=e2p[:, : gn * P],
                                             func=Act.Tanh)
                        wp_ = ppB.tile([P, 4 * P], f32, space="PSUM",
                                       tag="big")
                        for j in range(gn):
                            nc.tensor.matmul(
                                out=wp_[:, j * P: (j + 1) * P],
                                lhsT=e2T[:, j * P: (j + 1) * P],
                                rhs=fTb[:], start=True, stop=True)
                        for j in range(gn):
                            i = g0 + j
                            wsel = xp2.tile([P, P], bf16, tag="wsel")
                            wc = sm2.tile([P, 1], f32, tag="wc")
                            nc.vector.scalar_tensor_tensor(
                                out=wsel[:], in0=iota_t[:],
                                scalar=tlt[:, i: i + 1],
                                in1=wp_[:, j * P: (j + 1) * P],
                                op0=Alu.is_equal, op1=Alu.mult,
                                accum_out=wc[:])
                            maskw = wk2.tile([P, P], bf16, tag="maskw")
                            nc.vector.tensor_scalar(
                                out=maskw[:], in0=iota_t[:],
                                scalar1=tlt[:, i: i + 1],
                                scalar2=wc[:],
                                op0=Alu.is_equal, op1=Alu.mult)
                            nc.tensor.matmul(out=outp, lhsT=maskw[:],
                                             rhs=ftg[:, i, :],
                                             start=(i == 0),
                                             stop=(i == TI2 - 1))
                def s2_c3(w2, st2):
                    s2_swb(w2, st2, range(0, 24, 4))

                def s2_c4(w2, st2):
                    s2_swb(w2, st2, range(24, TI2, 4))
                    outp = st2["outp"]
                    out_sb = wk2.tile([P, D], f32, tag="out_sb")
                    nc.vector.tensor_copy(out=out_sb[:], in_=outp)
                    nc.scalar.dma_start(out=outd[w2 * WS2: (w2 + 1) * WS2, :],
                                        in_=out_sb[:])

                chunks = [s2_c0, s2_c1, s2_c2, s2_c3, s2_c4]
                sched = {}          # batch -> list of (chunk_fn, w2)
                states = [dict() for _ in range(W2)]
                for w2 in range(W2):
                    for ci, fn in enumerate(chunks):
                        sched.setdefault(cuts[w2] + ci, []).append((fn, w2))
                pre = s1_load(0)
                for b in range(B1):
                    cur = pre
                    if b + 1 < B1:
                        pre = s1_load(b + 1)   # prefetch next batch's idx
                    s1_batch(b, cur)
                    for fn, w2 in sched.get(b, []):
                        fn(w2, states[w2])
                for b in range(B1, max(sched) + 1):
                    for fn, w2 in sched.get(b, []):
                        fn(w2, states[w2])
    nc.compile()
    return nc


def make_in_maps(dims, cores, pi_w, q_w, r_w):
    iota_bf = np.tile(np.arange(P, dtype=np.float32), (P, 1)).astype(BF16)
    ident = np.eye(P, dtype=np.float32)
    in_maps = []
    for c in range(NCORES):
        st = cores[c]
        in_maps.append({
            "hv2": st["hv2"],
            "hvlTw": st["hvlTw"],
            "hpT": st["hpT"],
            "htT": st["htT"],
            "qw": np.ascontiguousarray(q_w, np.float32),
            "rw": np.ascontiguousarray(r_w, np.float32),
            "pic": np.ascontiguousarray(pi_w.reshape(D, 1), np.float32),
            "iotab": iota_bf, "ident": ident,
            "g2d": st["g2"], "segd": st["seg"],
            "ftgd": st["ftg"],
            "tgtlocd": st["tgtloc"], "rec2d": st["rec2"],
        })
    return in_maps


def unshard(dims, cores, results):
    NTGT = dims["NTGT"]
    out = np.zeros((NTGT, D), np.float32)
    for c in range(NCORES):
        st = cores[c]
        o = results[c]["out"]
        tw = st["twin"]
        for w2 in range(dims["W2"]):
            sel = tw[w2] >= 0
            out[tw[w2][sel]] = o[w2 * WS2: w2 * WS2 + WS2][sel]
    return out


def kernel(**inputs):
    from concourse.bass_utils import run_bass_kernel_spmd

    h_v = np.asarray(inputs["h_v"], np.float32)
    h_p = np.asarray(inputs["h_p"], np.float32)
    h_t = np.asarray(inputs["h_t"], np.float32)
    pi_w = np.asarray(inputs["pi_w"], np.float32)
    q_w = np.asarray(inputs["q_w"], np.float32)
    r_w = np.asarray(inputs["r_w"], np.float32)
    int_src = np.asarray(inputs["int_src"]).astype(np.int64)
    int_dst = np.asarray(inputs["int_dst"]).astype(np.int64)
    agg_src = np.asarray(inputs["agg_src"]).astype(np.int64)
    agg_dst = np.asarray(inputs["agg_dst"]).astype(np.int64)
    assert np.array_equal(agg_src, np.arange(agg_src.shape[0])), \
        "kernel assumes agg_src == arange (per problem spec fill)"

    dims, cores = preprocess(h_v, h_p, h_t, int_src, int_dst, agg_dst)
    nc = build_program(dims)
    global _LAST_NC
    _LAST_NC = nc
    in_maps = make_in_maps(dims, cores, pi_w, q_w, r_w)
    res = run_bass_kernel_spmd(nc, in_maps, core_ids=list(range(NCORES)))
    return unshard(dims, cores, res.results)
